# revision 16
# baseline (speedup 1.0000x reference)
"""Sharded top-1 KNN (retrieval) on 8 TRN2 NeuronCores via Bass/Tile.

v5 strategy (hardcoded for x[2048,24,16], X_train[65536,384], Y_train[65536,24,1]):
  - Shard X_train rows across 8 cores (8192 rows each), rows permuted so the
    2 rows of each folded pooled-column are adjacent in ||t||^2 order.
  - fp8(e4m3) full-K scoring: cross = x.t over all 384 dims per core, as one
    DoubleRow matmul (k-dims 0..255, 2 k-tiles per instruction) plus one plain
    fp8 matmul (k-dims 256..383) per 512-column chunk, accumulated in PSUM
    fp32.  TensorE is the bottleneck (~118us/core); fp8 DR measured at the
    same per-instruction cost as a plain matmul (157 TF/s effective).
  - Minimal drain: fold-2 only.  Per PSUM bank pair, ScalarE casts the even
    bank to bf16 and VectorE does one tensor_tensor(max) of the odd bank
    against it -- one PSUM read per score, no merge tree, no on-device top-k.
    The fold-2 map [2048, 4096] bf16 per core (16 MB) is DMA'd out under the
    matmul shadow.
  - Host subtracts the shared per-pooled-column bias (mean ||t||^2/2 of the 2
    tt-adjacent rows), takes top-16 pooled columns per core, expands 8 cores
    x top-16 x 2 rows = 256 candidates per query, recomputes exact distances
    (fp32 prefilter -> float64 on the top 8), and returns Y_train[argmin]
    (ties: smallest global index, matching jnp.argmin).
  Max-pooling cannot hurt candidate recall (pooled-rank <= raw-rank); on this
  dataset the true NN's pooled rank is <= 2 everywhere vs the 16 kept.
"""

import os
import sys

import numpy as np

for _p in ("/opt/trn_rl_repo",):
    if os.path.isdir(_p) and _p not in sys.path:
        sys.path.insert(0, _p)

import ml_dtypes  # noqa: E402

B, T, F = 2048, 24, 16
D = T * F  # 384
N = 65536
NCORES = 8
NS = N // NCORES  # 8192 rows per core
MT = B // 128  # 16 query tiles
NCHUNK = 512
NT = NS // NCHUNK  # 16 train chunks per core
FOLD = 2  # chunks max-folded into one pooled column
NP2 = NS // FOLD  # 4096 pooled positions
TOPK = 16
KDR = 256  # k-dims covered by the DoubleRow matmul

_BF16 = ml_dtypes.bfloat16
_F8 = ml_dtypes.float8_e4m3fn


def build_nc(b=B, ns=NS):
    """Per-core Bass program (SPMD: same program, per-core inputs)."""
    import concourse.tile as tile
    from concourse import bacc, mybir

    mt = b // 128

    nc = bacc.Bacc(None, target_bir_lowering=False)
    xdr = nc.dram_tensor("xdr", [128, 2, b], mybir.dt.float8e4, kind="ExternalInput")
    xk2 = nc.dram_tensor("xk2", [128, b], mybir.dt.float8e4, kind="ExternalInput")
    Xdr = nc.dram_tensor("Xdr", [128, 2, ns], mybir.dt.float8e4, kind="ExternalInput")
    Xk2 = nc.dram_tensor("Xk2", [128, ns], mybir.dt.float8e4, kind="ExternalInput")
    pool_out = nc.dram_tensor("pool", [b, NP2], mybir.dt.bfloat16, kind="ExternalOutput")

    with tile.TileContext(nc) as tc:
        with (
            tc.tile_pool(name="wpool", bufs=1) as wpool,
            tc.tile_pool(name="ppool", bufs=8, space="PSUM") as ppool,
            tc.tile_pool(name="cpool", bufs=8) as cpool,
            tc.tile_pool(name="vpool", bufs=2) as vpool,
        ):
            # input DMAs on the SP queue, ordered so the first half-m-tile's
            # dependencies stream in first (HBM-bound startup ~4us, not 11us)
            xdr_sb = wpool.tile([128, 2, b], mybir.dt.float8e4, name="xdr_sb", tag="xdr")
            xk2_sb = wpool.tile([128, b], mybir.dt.float8e4, name="xk2_sb", tag="xk2")
            Xdr_sb = wpool.tile([128, 2, ns], mybir.dt.float8e4, name="Xdr_sb", tag="Xdr")
            Xk2_sb = wpool.tile([128, ns], mybir.dt.float8e4, name="Xk2_sb", tag="Xk2")
            # DR-side inputs stream on the SP queue, k2-side on the Act queue
            # (both HWDGE) -- parallel issue, ~620ns/DMA instruction
            en = ns // 8
            nc.sync.dma_start(xdr_sb[:, :, :128], xdr[:, :, :128])  # m0 weights
            nc.sync.dma_start(Xdr_sb[:, :, :en], Xdr[:, :, :en])
            nc.sync.dma_start(Xdr_sb[:, :, en : 2 * en], Xdr[:, :, en : 2 * en])
            nc.sync.dma_start(xdr_sb[:, :, 128:], xdr[:, :, 128:])
            nc.scalar.dma_start(xk2_sb[:, :128], xk2[:, :128])
            nc.scalar.dma_start(Xk2_sb[:, :en], Xk2[:, :en])
            nc.scalar.dma_start(Xk2_sb[:, en : 2 * en], Xk2[:, en : 2 * en])
            nc.scalar.dma_start(xk2_sb[:, 128:], xk2[:, 128:])
            for qq in range(1, 4):  # quarters for the rest
                sl = slice(qq * 2 * en, (qq + 1) * 2 * en)
                nc.sync.dma_start(Xdr_sb[:, :, sl], Xdr[:, :, sl])
                nc.scalar.dma_start(Xk2_sb[:, sl], Xk2[:, sl])


            for m in range(mt):
                ms = slice(m * 128, (m + 1) * 128)
                vout = vpool.tile([128, NP2], mybir.dt.bfloat16, name="vout")
                for h in range(2):  # 8 chunks per half (one PSUM rotation)
                    pss = [
                        ppool.tile([128, NCHUNK], mybir.dt.float32, name="ps", tag="ps")
                        for _ in range(8)
                    ]
                    # DoubleRow pass (k 0..255), shared stationary weights
                    for j in range(8):
                        n = h * 8 + j
                        nc.tensor.matmul(
                            pss[j][:],
                            xdr_sb[:, :, ms],
                            Xdr_sb[:, :, n * NCHUNK : (n + 1) * NCHUNK],
                            perf_mode=mybir.MatmulPerfMode.DoubleRow,
                            start=True,
                            stop=False,
                        )
                    # plain fp8 pass (k 256..383)
                    for j in range(8):
                        n = h * 8 + j
                        nc.tensor.matmul(
                            pss[j][:],
                            xk2_sb[:, ms],
                            Xk2_sb[:, n * NCHUNK : (n + 1) * NCHUNK],
                            start=False,
                            stop=True,
                        )
                    # fold-2 drain: per bank pair, ScalarE casts the even bank,
                    # VectorE maxes the odd bank against it (one PSUM read per
                    # score, no merge tree)
                    for q in range(4):
                        cq = cpool.tile(
                            [128, NCHUNK], mybir.dt.bfloat16, name="cq", tag=f"cq{q}"
                        )
                        nc.scalar.copy(cq[:], pss[2 * q][:])
                        g = 4 * h + q
                        nc.vector.tensor_tensor(
                            vout[:, g * NCHUNK : (g + 1) * NCHUNK],
                            pss[2 * q + 1][:],
                            cq[:],
                            op=mybir.AluOpType.max,
                        )
                    nc.sync.dma_start(
                        pool_out[ms, h * 4 * NCHUNK : (h + 1) * 4 * NCHUNK],
                        vout[:, h * 4 * NCHUNK : (h + 1) * 4 * NCHUNK],
                    )
    nc.finalize()  # Bacc register allocation; walrus rejects unfinalized BIR
    return nc


_NC = None


def _get_nc():
    global _NC
    if _NC is None:
        _NC = build_nc()
    return _NC


def _shard_perm(tt, ns):
    """Device row n = ch*NCHUNK + j; pooled-2 column p = (ch//2)*NCHUNK + j
    covers chunks {2g, 2g+1}.  Give row n sorted rank p*2 + (ch%2) so each
    pooled column's 2 rows are tt-adjacent."""
    order = np.argsort(tt, kind="stable")  # sorted rank -> original row
    r = np.arange(ns)
    p, i = r // FOLD, r % FOLD
    g, j = p // NCHUNK, p % NCHUNK
    devrow = (g * FOLD + i) * NCHUNK + j
    perm = np.empty(ns, dtype=np.int64)
    perm[devrow] = order[r]
    return perm  # device row n holds original row perm[n]


def _prep_in_maps(xf, X_train):
    x8 = xf.astype(_F8)  # [B, D]
    xdr = np.ascontiguousarray(
        x8[:, :KDR].T.reshape(2, 128, B).transpose(1, 0, 2)
    )  # [128, 2, B]
    xk2 = np.ascontiguousarray(x8[:, KDR:].T)  # [128, B]
    in_maps = []
    perms = []
    ttfs = []
    for c in range(NCORES):
        Xs = X_train[c * NS : (c + 1) * NS]
        tt = (Xs.astype(np.float64) ** 2).sum(axis=1)
        perm = _shard_perm(tt, NS)
        perms.append(perm)
        X8 = Xs[perm].astype(_F8)  # [NS, D]
        Xdr = np.ascontiguousarray(
            X8[:, :KDR].T.reshape(2, 128, NS).transpose(1, 0, 2)
        )  # [128, 2, NS]
        Xk2 = np.ascontiguousarray(X8[:, KDR:].T)  # [128, NS]
        # shared bias per pooled column = mean tt/2 of its 2 folded rows
        tt_dev = tt[perm] * 0.5
        ttf = tt_dev.reshape(NT // FOLD, FOLD, NCHUNK).mean(axis=1).reshape(NP2)
        ttfs.append(ttf.astype(np.float32))
        in_maps.append({"xdr": xdr, "xk2": xk2, "Xdr": Xdr, "Xk2": Xk2})
    return in_maps, perms, ttfs


def _refine(xf, X_train, Y_train, cand):
    """cand: [B, C] global candidate row indices (sorted ascending, unique)."""
    b, C = cand.shape
    x32 = xf.astype(np.float32)
    keep = 8
    top = np.empty((b, keep), dtype=np.int64)
    step = 256
    for s in range(0, b, step):
        e = min(s + step, b)
        Xc = X_train[cand[s:e]]  # [q, C, D] fp32 gather
        diff = x32[s:e, None, :] - Xc
        d2 = np.einsum("qcd,qcd->qc", diff, diff)
        sel = np.argpartition(d2, keep, axis=1)[:, :keep]
        top[s:e] = np.take_along_axis(cand[s:e], sel, axis=1)
    # exact float64 pass on the 8 survivors; ties -> smallest global index
    top = np.sort(top, axis=1)
    xd = xf.astype(np.float64)
    Xt = X_train[top].astype(np.float64)  # [B, 8, D]
    diff = xd[:, None, :] - Xt
    d2 = np.einsum("qcd,qcd->qc", diff, diff)
    best = top[np.arange(b), np.argmin(d2, axis=1)]
    return Y_train[best].astype(np.float32)


def kernel(x, X_train, Y_train, _trace=False, _tmpdir=None):
    from concourse.bass_utils import run_bass_kernel_spmd

    x = np.asarray(x, dtype=np.float32)
    X_train = np.asarray(X_train, dtype=np.float32)
    Y_train = np.asarray(Y_train, dtype=np.float32)
    xf = x.reshape(B, D)

    in_maps, perms, ttfs = _prep_in_maps(xf, X_train)
    nc = _get_nc()
    kw = {}
    if _trace:
        kw = {"trace": True, "tmpdir": _tmpdir}
    res = run_bass_kernel_spmd(nc, in_maps, core_ids=list(range(NCORES)), **kw)

    # host: bias + top-16 pooled columns per core -> 256 candidates/query
    cands = []
    for c in range(NCORES):
        maps = res.results[c]["pool"].astype(np.float32)  # [B, NP2]
        score = maps - ttfs[c][None, :]
        pcol = np.argpartition(-score, TOPK, axis=1)[:, :TOPK]  # [B, 16]
        g, j = pcol // NCHUNK, pcol % NCHUNK
        devrows = (
            (g[:, :, None] * FOLD + np.arange(FOLD)[None, None, :]) * NCHUNK
            + j[:, :, None]
        ).reshape(B, TOPK * FOLD)
        cands.append(perms[c][devrows] + c * NS)
    cand = np.sort(np.concatenate(cands, axis=1), axis=1)  # [B, 256]
    out = _refine(xf, X_train, Y_train, cand)
    if _trace:
        return out, res
    return out
